# revision 12
# baseline (speedup 1.0000x reference)
"""DistMaps Trainium2 kernel (saturation-sparse).

tanh(2*sqrt(d2)) rounds to exactly 1.0 in fp32 for d2 >= 18.75, so only
pixels within sqrt(20)*5 ~ 22.4 px of a valid click can differ from 1.0.
Per-(group, row-block) accumulators are initialized to 22500 (saturated)
and, per click, only the [row-block] x [cols pc +/- 22.4] window is
produced (K=2 matmul on PE -> fp32 PSUM chunk) and min-accumulated on
the DVE directly from PSUM into fp32 accumulators.  Finals per group:
sqrt then tanh(2x) on ScalarE (batched by activation-table set), then
quartered DMAs out — pipelined with later chunks.

Host-side prep (all O(P2*W) = 24K elements, 0.6% of the output size):
the 1-D squared-distance lookup tables rowsq[pt, r] = ((r - pr)/s)^2 and
colsq[pt, c] = ((c - pc)/s)^2 (invalid clicks' rows forced to BIG^2) are
computed in numpy and DMA'd in as two [2, P2*W] fp16 tables whose other
row is ones — the K=2 chunk matmul reads (rowsq; ones) as lhsT and
(ones; colsq) as rhs.  All 4.2M output pixels are produced on-device.

Chunk lists are input-dependent and differ per batch, so each core gets
its own specialized program; the 8 programs are dispatched concurrently
onto their own NeuronCores via the PJRT path (async jax dispatch).
Excluded-by-construction chunks can only produce d2 > 20, whose output
rounds to 1.0 on both sides, so results match the dense reference.
"""

import sys

for _p in ("/opt/trn_rl_repo", "/root/.axon_site/_ro/trn_rl_repo"):
    if _p not in sys.path:
        sys.path.append(_p)

import math

import numpy as np

import concourse.bass as bass
from concourse import bacc
import concourse.mybir as mybir
from concourse.tile import TileContext

B, C, H, W = 8, 3, 512, 512
P2 = 48
PG = 24
NCORES = 8
SCALE = 5.0
INV_S = 1.0 / SCALE
BIG = 150.0
ACC_INIT = 22500.0   # = BIG^2; saturates tanh(2*sqrt(.)) to 1.0
D2_THRESH = 20.0     # include margin over the 18.75 fp32 saturation point
COL_HALF = SCALE * math.sqrt(D2_THRESH)  # 22.36 px
FL = P2 * W

FP32 = mybir.dt.float32
FP16 = mybir.dt.float16


def chunk_plan(coords_b: np.ndarray):
    """Chunk list [(g, q, pt, lo, hi)] for one batch's coords."""
    chunks = []
    for g in range(2):
        for j in range(PG):
            pt = g * PG + j
            pr, pc = float(coords_b[pt, 0]), float(coords_b[pt, 1])
            if max(pr, pc) < 0:
                continue  # invalid click
            lo = max(0, int(math.floor(pc - COL_HALF)))
            hi = min(W, int(math.ceil(pc + COL_HALF)) + 1)
            if lo >= hi:
                continue  # column window off-image
            for q in range(4):
                r0, r1 = q * 128, q * 128 + 127
                dr = 0.0 if r0 <= pr <= r1 else min(abs(pr - r0), abs(pr - r1))
                if (dr * INV_S) ** 2 <= D2_THRESH:
                    chunks.append((g, q, pt, lo, hi))
    return chunks


def host_tables(coords_b: np.ndarray):
    """[2, FL] fp16 tables: (rowsq_flat; ones) and (ones; colsq_flat)."""
    pts = coords_b[:, :2].astype(np.float64)
    invalid = pts.max(axis=1) < 0
    x = np.arange(W, dtype=np.float64)
    raff = (x[None, :] - pts[:, 0:1]) * INV_S
    raff[invalid] = BIG  # saturate invalid clicks via the row term
    caff = (x[None, :] - pts[:, 1:2]) * INV_S
    rowsq = (raff * raff).astype(np.float16).reshape(-1)
    colsq = (caff * caff).astype(np.float16).reshape(-1)
    ones = np.ones_like(rowsq)
    tab_r = np.stack([rowsq, ones])
    tab_c = np.stack([ones, colsq])
    return tab_r, tab_c


def build_program(chunks):
    nc = bacc.Bacc("TRN2", num_devices=1, debug=False)

    QFL = FL // 4  # 12 clicks per table piece
    tab_r = [
        nc.dram_tensor(f"tab_r{h}", [2, QFL], FP16, kind="ExternalInput")
        for h in range(4)
    ]
    tab_c = [
        nc.dram_tensor(f"tab_c{h}", [2, QFL], FP16, kind="ExternalInput")
        for h in range(4)
    ]
    out = nc.dram_tensor("out", [2, H, W], FP32, kind="ExternalOutput")

    with TileContext(nc) as tc:
        with (
            tc.tile_pool(name="const", bufs=1) as constp,
            tc.tile_pool(name="flats", bufs=1) as flatp,
            tc.tile_pool(name="accp", bufs=1) as accp,
            tc.tile_pool(name="outp", bufs=2) as outp,
            tc.tile_pool(name="pschunk", bufs=8, space="PSUM") as pscp,
        ):
            # flat tables straight from HBM in 12-click pieces (8 parallel
            # DMAs) so the first chunks start as soon as piece 0 arrives
            flatrow = [
                flatp.tile([2, FL // 4], FP16, tag=f"flatrow{h}", name=f"flatrow{h}")
                for h in range(4)
            ]
            flatcol = [
                flatp.tile([2, FL // 4], FP16, tag=f"flatcol{h}", name=f"flatcol{h}")
                for h in range(4)
            ]
            for h in range(4):
                nc.sync.dma_start(flatrow[h][:], tab_r[h][:, :])
                nc.sync.dma_start(flatcol[h][:], tab_c[h][:, :])

            # warm the sqrt table set at t=0 (the sqrt batch then needs no
            # load; sets are not evicted until the first tanh)
            scratch = constp.tile([1, 16], FP32, tag="scratch")
            warm = constp.tile([1, 16], FP32, tag="warm")
            nc.gpsimd.memset(scratch[:], 1.0)
            nc.scalar.activation(warm[:], scratch[:], mybir.ActivationFunctionType.Sqrt)

            # per-(group, row-block) accumulators, init on idle GPSIMD
            acc = {}
            for g in range(2):
                for q in range(4):
                    acc[(g, q)] = accp.tile(
                        [128, W], FP32, tag=f"acc{g}{q}", name=f"acc{g}{q}"
                    )
                    nc.gpsimd.memset(acc[(g, q)][:], ACC_INIT)

            out_v = out.rearrange("t (q p) u -> t p q u", p=128)
            by_gq = {}
            for (cg, q, pt, lo, hi) in chunks:
                by_gq.setdefault((cg, q), []).append((pt, lo, hi))
            sqs = [
                outp.tile([128, 2048], FP32, tag=f"sqg{g}", name=f"sqg{g}")
                for g in range(2)
            ]

            for g in range(2):
                for q in range(4):
                    # within a block, process earlier-arriving table pieces first
                    for (pt, lo, hi) in sorted(
                        by_gq.get((g, q), []), key=lambda t: t[0] // 12
                    ):
                        w = hi - lo
                        ch = pscp.tile([128, 64], FP32, tag="chunk", name="ch")
                        # d2 = rowsq[pt, block] (x) ones + ones (x) colsq[pt, lo:hi]
                        h, j = pt // 12, pt % 12
                        nc.tensor.matmul(
                            ch[:, :w],
                            flatrow[h][:, j * W + q * 128 : j * W + (q + 1) * 128],
                            flatcol[h][:, j * W + lo : j * W + hi],
                            start=True,
                            stop=True,
                        )
                        dst = acc[(g, q)][:, lo:hi]
                        nc.vector.tensor_tensor(dst, dst, ch[:, :w], mybir.AluOpType.min)

                    # sqrt inline per block: starts as soon as this block's
                    # chunks are done (one table set across the whole loop)
                    nc.scalar.activation(
                        sqs[g][:, q * W : (q + 1) * W],
                        acc[(g, q)][:],
                        mybir.ActivationFunctionType.Sqrt,
                    )

                # tanh + DMA per group (the whole-tile read of sqs[g]
                # already orders it after all four of the group's sqrts)
                res = outp.tile([128, 2048], FP32, tag=f"res{g}", name=f"res{g}")
                nc.scalar.activation(
                    res[:], sqs[g][:], mybir.ActivationFunctionType.Tanh, scale=2.0
                )
                res_v = res.rearrange("p (q u) -> p q u", u=W)
                for q in range(4):
                    nc.sync.dma_start(out_v[g, :, q], res_v[:, q])

    nc.finalize()
    return nc


# ---------------------------------------------------------------------------
# Per-core concurrent execution: each core gets its own specialized NEFF,
# dispatched asynchronously onto its own device (modeled on
# bass2jax.run_bass_via_pjrt's single-core path).
# ---------------------------------------------------------------------------


def _make_exec(nc):
    import jax
    from concourse.bass2jax import _bass_exec_p, install_neuronx_cc_hook
    import concourse.mybir as mb

    install_neuronx_cc_hook()

    pid_name = nc.partition_id_tensor.name if nc.partition_id_tensor else None
    in_names, out_names, out_avals, zero_outs = [], [], [], []
    pid_shape_dtype = None
    for alloc in nc.m.functions[0].allocations:
        if not isinstance(alloc, mb.MemoryLocationSet):
            continue
        name = alloc.memorylocations[0].name
        if alloc.kind == "ExternalInput":
            if name == pid_name:
                pid_shape_dtype = (tuple(alloc.tensor_shape), mb.dt.np(alloc.dtype))
            in_names.append(name)
        elif alloc.kind == "ExternalOutput":
            out_names.append(name)
            shape = tuple(alloc.tensor_shape)
            dtype = mb.dt.np(alloc.dtype)
            out_avals.append(jax.core.ShapedArray(shape, dtype))
            zero_outs.append(np.zeros(shape, dtype))
    n_params = len(in_names)
    all_names = in_names + out_names

    def _body(*args):
        outs = _bass_exec_p.bind(
            *args,
            out_avals=tuple(out_avals),
            in_names=tuple(all_names),
            out_names=tuple(out_names),
            lowering_input_output_aliases=(),
            sim_require_finite=True,
            sim_require_nnan=True,
            nc=nc,
        )
        return tuple(outs)

    donate = tuple(range(n_params, n_params + len(out_names)))
    jitted = jax.jit(_body, donate_argnums=donate, keep_unused=True)
    extra = (pid_name, pid_shape_dtype) if pid_name is not None else None
    return jitted, in_names[:n_params], out_names, zero_outs, extra


_CACHE: dict = {}


def kernel(x: np.ndarray, coords: np.ndarray) -> np.ndarray:
    import jax

    coords = np.asarray(coords, dtype=np.float32)
    devices = jax.devices()[:NCORES]

    futures = []
    for b in range(NCORES):
        plan = tuple(chunk_plan(coords[b]))
        entry = _CACHE.get(plan)
        if entry is None:
            nc = build_program(list(plan))
            entry = _make_exec(nc)
            _CACHE[plan] = entry
        jitted, in_names, out_names, zero_outs, extra = entry
        tab_r, tab_c = host_tables(coords[b])
        qf = FL // 4
        in_map = {}
        for h in range(4):
            in_map[f"tab_r{h}"] = np.ascontiguousarray(tab_r[:, h * qf : (h + 1) * qf])
            in_map[f"tab_c{h}"] = np.ascontiguousarray(tab_c[:, h * qf : (h + 1) * qf])
        if extra is not None:
            in_map[extra[0]] = np.full(extra[1][0], b, dtype=extra[1][1])
        args = [jax.device_put(in_map[n], devices[b]) for n in in_names]
        args += [jax.device_put(z.copy(), devices[b]) for z in zero_outs]
        futures.append((out_names, jitted(*args)))

    outs = []
    for out_names, arrs in futures:
        res = {n: np.asarray(a) for n, a in zip(out_names, arrs)}
        outs.append(res["out"].reshape(2, H, W))
    return np.stack(outs, axis=0)


# revision 13
# speedup vs baseline: 1.0119x; 1.0119x over previous
"""DistMaps Trainium2 kernel (saturation-sparse).

tanh(2*sqrt(d2)) rounds to exactly 1.0 in fp32 for d2 >= 18.75, so only
pixels within sqrt(20)*5 ~ 22.4 px of a valid click can differ from 1.0.
Per-(group, row-block) accumulators are initialized to 22500 (saturated)
and, per click, only the [row-block] x [cols pc +/- 22.4] window is
produced (K=2 matmul on PE -> fp32 PSUM chunk) and min-accumulated on
the DVE directly from PSUM into fp32 accumulators.  Finals per group:
sqrt then tanh(2x) on ScalarE (batched by activation-table set), then
quartered DMAs out — pipelined with later chunks.

Host-side prep (all O(P2*W) = 24K elements, 0.6% of the output size):
the 1-D squared-distance lookup tables rowsq[pt, r] = ((r - pr)/s)^2 and
colsq[pt, c] = ((c - pc)/s)^2 (invalid clicks' rows forced to BIG^2) are
computed in numpy and DMA'd in as two [2, P2*W] fp16 tables whose other
row is ones — the K=2 chunk matmul reads (rowsq; ones) as lhsT and
(ones; colsq) as rhs.  All 4.2M output pixels are produced on-device.

Chunk lists are input-dependent and differ per batch, so each core gets
its own specialized program; the 8 programs are dispatched concurrently
onto their own NeuronCores via the PJRT path (async jax dispatch).
Excluded-by-construction chunks can only produce d2 > 20, whose output
rounds to 1.0 on both sides, so results match the dense reference.
"""

import sys

for _p in ("/opt/trn_rl_repo", "/root/.axon_site/_ro/trn_rl_repo"):
    if _p not in sys.path:
        sys.path.append(_p)

import math

import numpy as np

import concourse.bass as bass
from concourse import bacc
import concourse.mybir as mybir
from concourse.tile import TileContext

B, C, H, W = 8, 3, 512, 512
P2 = 48
PG = 24
NCORES = 8
SCALE = 5.0
INV_S = 1.0 / SCALE
BIG = 150.0
ACC_INIT = 22500.0   # = BIG^2; saturates tanh(2*sqrt(.)) to 1.0
D2_THRESH = 20.0     # include margin over the 18.75 fp32 saturation point
COL_HALF = SCALE * math.sqrt(D2_THRESH)  # 22.36 px
FL = P2 * W

FP32 = mybir.dt.float32
FP16 = mybir.dt.float16


def chunk_plan(coords_b: np.ndarray):
    """Chunk list [(g, q, pt, lo, hi)] for one batch's coords."""
    chunks = []
    for g in range(2):
        for j in range(PG):
            pt = g * PG + j
            pr, pc = float(coords_b[pt, 0]), float(coords_b[pt, 1])
            if max(pr, pc) < 0:
                continue  # invalid click
            lo = max(0, int(math.floor(pc - COL_HALF)))
            hi = min(W, int(math.ceil(pc + COL_HALF)) + 1)
            if lo >= hi:
                continue  # column window off-image
            for q in range(4):
                r0, r1 = q * 128, q * 128 + 127
                dr = 0.0 if r0 <= pr <= r1 else min(abs(pr - r0), abs(pr - r1))
                if (dr * INV_S) ** 2 <= D2_THRESH:
                    chunks.append((g, q, pt, lo, hi))
    return chunks


def host_tables(coords_b: np.ndarray):
    """[2, FL] fp16 tables: (rowsq_flat; ones) and (ones; colsq_flat)."""
    pts = coords_b[:, :2].astype(np.float64)
    invalid = pts.max(axis=1) < 0
    x = np.arange(W, dtype=np.float64)
    raff = (x[None, :] - pts[:, 0:1]) * INV_S
    raff[invalid] = BIG  # saturate invalid clicks via the row term
    caff = (x[None, :] - pts[:, 1:2]) * INV_S
    rowsq = (raff * raff).astype(np.float16).reshape(-1)
    colsq = (caff * caff).astype(np.float16).reshape(-1)
    ones = np.ones_like(rowsq)
    tab_r = np.stack([rowsq, ones])
    tab_c = np.stack([ones, colsq])
    return tab_r, tab_c


def build_program(chunks):
    nc = bacc.Bacc("TRN2", num_devices=1, debug=False)

    HFL = FL // 2
    tab_r = [
        nc.dram_tensor(f"tab_r{g}", [2, HFL], FP16, kind="ExternalInput")
        for g in range(2)
    ]
    tab_c = [
        nc.dram_tensor(f"tab_c{g}", [2, HFL], FP16, kind="ExternalInput")
        for g in range(2)
    ]
    out = nc.dram_tensor("out", [2, H, W], FP32, kind="ExternalOutput")

    with TileContext(nc) as tc:
        with (
            tc.tile_pool(name="const", bufs=1) as constp,
            tc.tile_pool(name="flats", bufs=1) as flatp,
            tc.tile_pool(name="accp", bufs=1) as accp,
            tc.tile_pool(name="outp", bufs=2) as outp,
            tc.tile_pool(name="pschunk", bufs=8, space="PSUM") as pscp,
        ):
            # flat tables straight from HBM, split per group so group-0
            # chunks start as soon as its half arrives (4 parallel DMAs)
            flatrow = [
                flatp.tile([2, FL // 2], FP16, tag=f"flatrow{g}", name=f"flatrow{g}")
                for g in range(2)
            ]
            flatcol = [
                flatp.tile([2, FL // 2], FP16, tag=f"flatcol{g}", name=f"flatcol{g}")
                for g in range(2)
            ]
            for g in range(2):
                nc.sync.dma_start(flatrow[g][:], tab_r[g][:, :])
                nc.sync.dma_start(flatcol[g][:], tab_c[g][:, :])

            # warm the sqrt table set at t=0 (the sqrt batch then needs no
            # load; sets are not evicted until the first tanh)
            scratch = constp.tile([1, 16], FP32, tag="scratch")
            warm = constp.tile([1, 16], FP32, tag="warm")
            nc.gpsimd.memset(scratch[:], 1.0)
            nc.scalar.activation(warm[:], scratch[:], mybir.ActivationFunctionType.Sqrt)

            # per-(group, row-block) accumulators, init on idle GPSIMD
            acc = {}
            for g in range(2):
                for q in range(4):
                    acc[(g, q)] = accp.tile(
                        [128, W], FP32, tag=f"acc{g}{q}", name=f"acc{g}{q}"
                    )
                    nc.gpsimd.memset(acc[(g, q)][:], ACC_INIT)

            out_v = out.rearrange("t (q p) u -> t p q u", p=128)
            by_gq = {}
            for (cg, q, pt, lo, hi) in chunks:
                by_gq.setdefault((cg, q), []).append((pt, lo, hi))
            sqs = [
                outp.tile([128, 2048], FP32, tag=f"sqg{g}", name=f"sqg{g}")
                for g in range(2)
            ]

            for g in range(2):
                for q in range(4):
                    for (pt, lo, hi) in by_gq.get((g, q), []):
                        w = hi - lo
                        ch = pscp.tile([128, 64], FP32, tag="chunk", name="ch")
                        # d2 = rowsq[pt, block] (x) ones + ones (x) colsq[pt, lo:hi]
                        j = pt - g * PG
                        nc.tensor.matmul(
                            ch[:, :w],
                            flatrow[g][:, j * W + q * 128 : j * W + (q + 1) * 128],
                            flatcol[g][:, j * W + lo : j * W + hi],
                            start=True,
                            stop=True,
                        )
                        dst = acc[(g, q)][:, lo:hi]
                        nc.vector.tensor_tensor(dst, dst, ch[:, :w], mybir.AluOpType.min)

                    # sqrt inline per block: starts as soon as this block's
                    # chunks are done (one table set across the whole loop)
                    nc.scalar.activation(
                        sqs[g][:, q * W : (q + 1) * W],
                        acc[(g, q)][:],
                        mybir.ActivationFunctionType.Sqrt,
                    )

                # tanh + DMA per group (the whole-tile read of sqs[g]
                # already orders it after all four of the group's sqrts)
                res = outp.tile([128, 2048], FP32, tag=f"res{g}", name=f"res{g}")
                nc.scalar.activation(
                    res[:], sqs[g][:], mybir.ActivationFunctionType.Tanh, scale=2.0
                )
                res_v = res.rearrange("p (q u) -> p q u", u=W)
                for q in range(4):
                    nc.sync.dma_start(out_v[g, :, q], res_v[:, q])

    nc.finalize()
    return nc


# ---------------------------------------------------------------------------
# Per-core concurrent execution: each core gets its own specialized NEFF,
# dispatched asynchronously onto its own device (modeled on
# bass2jax.run_bass_via_pjrt's single-core path).
# ---------------------------------------------------------------------------


def _make_exec(nc):
    import jax
    from concourse.bass2jax import _bass_exec_p, install_neuronx_cc_hook
    import concourse.mybir as mb

    install_neuronx_cc_hook()

    pid_name = nc.partition_id_tensor.name if nc.partition_id_tensor else None
    in_names, out_names, out_avals, zero_outs = [], [], [], []
    pid_shape_dtype = None
    for alloc in nc.m.functions[0].allocations:
        if not isinstance(alloc, mb.MemoryLocationSet):
            continue
        name = alloc.memorylocations[0].name
        if alloc.kind == "ExternalInput":
            if name == pid_name:
                pid_shape_dtype = (tuple(alloc.tensor_shape), mb.dt.np(alloc.dtype))
            in_names.append(name)
        elif alloc.kind == "ExternalOutput":
            out_names.append(name)
            shape = tuple(alloc.tensor_shape)
            dtype = mb.dt.np(alloc.dtype)
            out_avals.append(jax.core.ShapedArray(shape, dtype))
            zero_outs.append(np.zeros(shape, dtype))
    n_params = len(in_names)
    all_names = in_names + out_names

    def _body(*args):
        outs = _bass_exec_p.bind(
            *args,
            out_avals=tuple(out_avals),
            in_names=tuple(all_names),
            out_names=tuple(out_names),
            lowering_input_output_aliases=(),
            sim_require_finite=True,
            sim_require_nnan=True,
            nc=nc,
        )
        return tuple(outs)

    donate = tuple(range(n_params, n_params + len(out_names)))
    jitted = jax.jit(_body, donate_argnums=donate, keep_unused=True)
    extra = (pid_name, pid_shape_dtype) if pid_name is not None else None
    return jitted, in_names[:n_params], out_names, zero_outs, extra


_CACHE: dict = {}


def kernel(x: np.ndarray, coords: np.ndarray) -> np.ndarray:
    import jax

    coords = np.asarray(coords, dtype=np.float32)
    devices = jax.devices()[:NCORES]

    futures = []
    for b in range(NCORES):
        plan = tuple(chunk_plan(coords[b]))
        entry = _CACHE.get(plan)
        if entry is None:
            nc = build_program(list(plan))
            entry = _make_exec(nc)
            _CACHE[plan] = entry
        jitted, in_names, out_names, zero_outs, extra = entry
        tab_r, tab_c = host_tables(coords[b])
        h = FL // 2
        in_map = {
            "tab_r0": np.ascontiguousarray(tab_r[:, :h]),
            "tab_r1": np.ascontiguousarray(tab_r[:, h:]),
            "tab_c0": np.ascontiguousarray(tab_c[:, :h]),
            "tab_c1": np.ascontiguousarray(tab_c[:, h:]),
        }
        if extra is not None:
            in_map[extra[0]] = np.full(extra[1][0], b, dtype=extra[1][1])
        args = [jax.device_put(in_map[n], devices[b]) for n in in_names]
        args += [jax.device_put(z.copy(), devices[b]) for z in zero_outs]
        futures.append((out_names, jitted(*args)))

    outs = []
    for out_names, arrs in futures:
        res = {n: np.asarray(a) for n, a in zip(out_names, arrs)}
        outs.append(res["out"].reshape(2, H, W))
    return np.stack(outs, axis=0)


# revision 15
# speedup vs baseline: 1.0194x; 1.0074x over previous
"""DistMaps Trainium2 kernel (saturation-sparse).

tanh(2*sqrt(d2)) rounds to exactly 1.0 in fp32 for d2 >= 18.75, so only
pixels within sqrt(20)*5 ~ 22.4 px of a valid click can differ from 1.0.
Per-(group, row-block) accumulators are initialized to 22500 (saturated)
and, per click, only the [row-block] x [cols pc +/- 22.4] window is
produced (K=2 matmul on PE -> fp32 PSUM chunk) and min-accumulated on
the DVE directly from PSUM into fp32 accumulators.  Finals per group:
sqrt then tanh(2x) on ScalarE (batched by activation-table set), then
quartered DMAs out — pipelined with later chunks.

Host-side prep (all O(P2*W) = 24K elements, 0.6% of the output size):
the 1-D squared-distance lookup tables rowsq[pt, r] = ((r - pr)/s)^2 and
colsq[pt, c] = ((c - pc)/s)^2 (invalid clicks' rows forced to BIG^2) are
computed in numpy and DMA'd in as two [2, P2*W] fp16 tables whose other
row is ones — the K=2 chunk matmul reads (rowsq; ones) as lhsT and
(ones; colsq) as rhs.  All 4.2M output pixels are produced on-device.

Chunk lists are input-dependent and differ per batch, so each core gets
its own specialized program; the 8 programs are dispatched concurrently
onto their own NeuronCores via the PJRT path (async jax dispatch).
Excluded-by-construction chunks can only produce d2 > 20, whose output
rounds to 1.0 on both sides, so results match the dense reference.
"""

import sys

for _p in ("/opt/trn_rl_repo", "/root/.axon_site/_ro/trn_rl_repo"):
    if _p not in sys.path:
        sys.path.append(_p)

import math

import numpy as np

import concourse.bass as bass
from concourse import bacc
import concourse.mybir as mybir
from concourse.tile import TileContext

B, C, H, W = 8, 3, 512, 512
P2 = 48
PG = 24
NCORES = 8
SCALE = 5.0
INV_S = 1.0 / SCALE
BIG = 150.0
ACC_INIT = 22500.0   # = BIG^2; saturates tanh(2*sqrt(.)) to 1.0
D2_THRESH = 20.0     # include margin over the 18.75 fp32 saturation point
COL_HALF = SCALE * math.sqrt(D2_THRESH)  # 22.36 px
FL = P2 * W

FP32 = mybir.dt.float32
FP16 = mybir.dt.float16


def chunk_plan(coords_b: np.ndarray):
    """Chunk list [(g, q, pt, lo, hi)] for one batch's coords."""
    chunks = []
    for g in range(2):
        for j in range(PG):
            pt = g * PG + j
            pr, pc = float(coords_b[pt, 0]), float(coords_b[pt, 1])
            if max(pr, pc) < 0:
                continue  # invalid click
            lo = max(0, int(math.floor(pc - COL_HALF)))
            hi = min(W, int(math.ceil(pc + COL_HALF)) + 1)
            if lo >= hi:
                continue  # column window off-image
            for q in range(4):
                r0, r1 = q * 128, q * 128 + 127
                dr = 0.0 if r0 <= pr <= r1 else min(abs(pr - r0), abs(pr - r1))
                if (dr * INV_S) ** 2 <= D2_THRESH:
                    chunks.append((g, q, pt, lo, hi))
    return chunks


def host_tables(coords_b: np.ndarray):
    """[2, FL] fp16 tables: (rowsq_flat; ones) and (ones; colsq_flat)."""
    pts = coords_b[:, :2].astype(np.float64)
    invalid = pts.max(axis=1) < 0
    x = np.arange(W, dtype=np.float64)
    raff = (x[None, :] - pts[:, 0:1]) * INV_S
    raff[invalid] = BIG  # saturate invalid clicks via the row term
    caff = (x[None, :] - pts[:, 1:2]) * INV_S
    rowsq = (raff * raff).astype(np.float16).reshape(-1)
    colsq = (caff * caff).astype(np.float16).reshape(-1)
    ones = np.ones_like(rowsq)
    tab_r = np.stack([rowsq, ones])
    tab_c = np.stack([ones, colsq])
    return tab_r, tab_c


def build_program(chunks):
    nc = bacc.Bacc("TRN2", num_devices=1, debug=False)

    HFL = FL // 2
    tab_r = [
        nc.dram_tensor(f"tab_r{g}", [2, HFL], FP16, kind="ExternalInput")
        for g in range(2)
    ]
    tab_c = [
        nc.dram_tensor(f"tab_c{g}", [2, HFL], FP16, kind="ExternalInput")
        for g in range(2)
    ]
    out = nc.dram_tensor("out", [2, H, W], FP32, kind="ExternalOutput")

    with TileContext(nc) as tc:
        with (
            tc.tile_pool(name="const", bufs=1) as constp,
            tc.tile_pool(name="flats", bufs=1) as flatp,
            tc.tile_pool(name="accp", bufs=1) as accp,
            tc.tile_pool(name="outp", bufs=2) as outp,
            tc.tile_pool(name="pschunk", bufs=8, space="PSUM") as pscp,
        ):
            # flat tables straight from HBM, split per group so group-0
            # chunks start as soon as its half arrives (4 parallel DMAs)
            flatrow = [
                flatp.tile([2, FL // 2], FP16, tag=f"flatrow{g}", name=f"flatrow{g}")
                for g in range(2)
            ]
            flatcol = [
                flatp.tile([2, FL // 2], FP16, tag=f"flatcol{g}", name=f"flatcol{g}")
                for g in range(2)
            ]
            for g in range(2):
                nc.sync.dma_start(flatrow[g][:], tab_r[g][:, :])
                nc.sync.dma_start(flatcol[g][:], tab_c[g][:, :])

            # warm the sqrt table set at t=0 (the sqrt batch then needs no
            # load; sets are not evicted until the first tanh)
            scratch = constp.tile([1, 16], FP32, tag="scratch")
            warm = constp.tile([1, 16], FP32, tag="warm")
            nc.gpsimd.memset(scratch[:], 1.0)
            nc.scalar.activation(warm[:], scratch[:], mybir.ActivationFunctionType.Sqrt)

            # per-(group, row-block) accumulators, init on idle GPSIMD
            acc = {}
            for g in range(2):
                for q in range(4):
                    acc[(g, q)] = accp.tile(
                        [128, W], FP32, tag=f"acc{g}{q}", name=f"acc{g}{q}"
                    )
                    nc.gpsimd.memset(acc[(g, q)][:], ACC_INIT)

            out_v = out.rearrange("t (q p) u -> t p q u", p=128)
            by_gq = {}
            for (cg, q, pt, lo, hi) in chunks:
                by_gq.setdefault((cg, q), []).append((pt, lo, hi))
            sqs = [
                outp.tile([128, 2048], FP32, tag=f"sqg{g}", name=f"sqg{g}")
                for g in range(2)
            ]

            for g in range(2):
                for q in range(4):
                    for (pt, lo, hi) in by_gq.get((g, q), []):
                        w = hi - lo
                        ch = pscp.tile([128, 64], FP32, tag="chunk", name="ch")
                        # d2 = rowsq[pt, block] (x) ones + ones (x) colsq[pt, lo:hi]
                        j = pt - g * PG
                        nc.tensor.matmul(
                            ch[:, :w],
                            flatrow[g][:, j * W + q * 128 : j * W + (q + 1) * 128],
                            flatcol[g][:, j * W + lo : j * W + hi],
                            start=True,
                            stop=True,
                        )
                        dst = acc[(g, q)][:, lo:hi]
                        nc.vector.tensor_tensor(dst, dst, ch[:, :w], mybir.AluOpType.min)

                    # sqrt inline per block: starts as soon as this block's
                    # chunks are done (one table set across the whole loop)
                    nc.scalar.activation(
                        sqs[g][:, q * W : (q + 1) * W],
                        acc[(g, q)][:],
                        mybir.ActivationFunctionType.Sqrt,
                    )

                # tanh + DMA per group (the whole-tile read of sqs[g]
                # already orders it after all four of the group's sqrts)
                res = outp.tile([128, 2048], FP32, tag=f"res{g}", name=f"res{g}")
                nc.scalar.activation(
                    res[:], sqs[g][:], mybir.ActivationFunctionType.Tanh, scale=2.0
                )
                res_v = res.rearrange("p (q u) -> p q u", u=W)
                nc.sync.dma_start(out_v[g, :, 0:2], res_v[:, 0:2])
                nc.sync.dma_start(out_v[g, :, 2:4], res_v[:, 2:4])

    nc.finalize()
    return nc


# ---------------------------------------------------------------------------
# Per-core concurrent execution: each core gets its own specialized NEFF,
# dispatched asynchronously onto its own device (modeled on
# bass2jax.run_bass_via_pjrt's single-core path).
# ---------------------------------------------------------------------------


def _make_exec(nc):
    import jax
    from concourse.bass2jax import _bass_exec_p, install_neuronx_cc_hook
    import concourse.mybir as mb

    install_neuronx_cc_hook()

    pid_name = nc.partition_id_tensor.name if nc.partition_id_tensor else None
    in_names, out_names, out_avals, zero_outs = [], [], [], []
    pid_shape_dtype = None
    for alloc in nc.m.functions[0].allocations:
        if not isinstance(alloc, mb.MemoryLocationSet):
            continue
        name = alloc.memorylocations[0].name
        if alloc.kind == "ExternalInput":
            if name == pid_name:
                pid_shape_dtype = (tuple(alloc.tensor_shape), mb.dt.np(alloc.dtype))
            in_names.append(name)
        elif alloc.kind == "ExternalOutput":
            out_names.append(name)
            shape = tuple(alloc.tensor_shape)
            dtype = mb.dt.np(alloc.dtype)
            out_avals.append(jax.core.ShapedArray(shape, dtype))
            zero_outs.append(np.zeros(shape, dtype))
    n_params = len(in_names)
    all_names = in_names + out_names

    def _body(*args):
        outs = _bass_exec_p.bind(
            *args,
            out_avals=tuple(out_avals),
            in_names=tuple(all_names),
            out_names=tuple(out_names),
            lowering_input_output_aliases=(),
            sim_require_finite=True,
            sim_require_nnan=True,
            nc=nc,
        )
        return tuple(outs)

    donate = tuple(range(n_params, n_params + len(out_names)))
    jitted = jax.jit(_body, donate_argnums=donate, keep_unused=True)
    extra = (pid_name, pid_shape_dtype) if pid_name is not None else None
    return jitted, in_names[:n_params], out_names, zero_outs, extra


_CACHE: dict = {}


def kernel(x: np.ndarray, coords: np.ndarray) -> np.ndarray:
    import time

    # transient NRT_EXEC_UNIT_UNRECOVERABLE flakes have been observed on the
    # first execution of a freshly compiled program; retry a couple of times
    last = None
    for attempt in range(3):
        try:
            return _kernel_once(x, coords)
        except Exception as e:  # jax.errors.JaxRuntimeError and friends
            last = e
            _CACHE.clear()
            time.sleep(2.0)
    raise last


def _kernel_once(x: np.ndarray, coords: np.ndarray) -> np.ndarray:
    import jax

    coords = np.asarray(coords, dtype=np.float32)
    devices = jax.devices()[:NCORES]

    futures = []
    for b in range(NCORES):
        plan = tuple(chunk_plan(coords[b]))
        entry = _CACHE.get(plan)
        if entry is None:
            nc = build_program(list(plan))
            entry = _make_exec(nc)
            _CACHE[plan] = entry
        jitted, in_names, out_names, zero_outs, extra = entry
        tab_r, tab_c = host_tables(coords[b])
        h = FL // 2
        in_map = {
            "tab_r0": np.ascontiguousarray(tab_r[:, :h]),
            "tab_r1": np.ascontiguousarray(tab_r[:, h:]),
            "tab_c0": np.ascontiguousarray(tab_c[:, :h]),
            "tab_c1": np.ascontiguousarray(tab_c[:, h:]),
        }
        if extra is not None:
            in_map[extra[0]] = np.full(extra[1][0], b, dtype=extra[1][1])
        args = [jax.device_put(in_map[n], devices[b]) for n in in_names]
        args += [jax.device_put(z.copy(), devices[b]) for z in zero_outs]
        futures.append((out_names, jitted(*args)))

    outs = []
    for out_names, arrs in futures:
        res = {n: np.asarray(a) for n, a in zip(out_names, arrs)}
        outs.append(res["out"].reshape(2, H, W))
    return np.stack(outs, axis=0)
